# revision 9
# baseline (speedup 1.0000x reference)
"""Trainium2 Bass kernel for nn_Decoder: beta = softmax(alphas @ W^T), kl_alpha.

Strategy (8 NeuronCores, vocab-sharded tensor parallel):
  - Host prep: alphas = mu + eps*exp(0.5*ls) (tiny, [2500,256]); transpose to
    [256, 2500] and pad rows to 2560; split into fp16 hi/lo pair. W is padded
    to 50176 vocab, transposed, column-sharded 8x6272, fp16 hi/lo split.
  - Device, per core: logits tile = ah@wh + al@wh + ah@wl (3-term fp16
    compensated product ~ fp32 precision, full fp16 matmul rate) into PSUM;
    ScalarE computes u = exp(logits + bias_row) PSUM->SBUF with fused
    per-row accumulation (row-sums). Per 256-row chunk, the [128,2] row-sums
    are AllReduced (add) across the 8 cores; VectorE multiplies the SBUF-
    resident u in place by 1/S and DMAs straight to the output shard. The
    u tiles never leave SBUF unnormalized, so HBM traffic is one output write.
  - bias_row = -(sampled row max) from a strided host-side sample; any
    row bias within ~(-87,+87) of the true row max is exact (fp32 range),
    and a host-side finite check retries with a bumped bias if ever violated.
  - KL is computed on-device from muT/lsT/alphasT (elementwise + reductions),
    chunked so it fills VectorE idle slots; the data-independent constant
    part of the KL sum is added back on the host.
"""

import numpy as np
import ml_dtypes

import concourse.bacc as bacc
import concourse.mybir as mybir
import concourse.tile as tile
from concourse import bass_utils

F32 = mybir.dt.float32
F16 = mybir.dt.float16
X = mybir.AxisListType.X
EXP = mybir.ActivationFunctionType.Exp

N_CORES = 8
T, K, R, V = 50, 50, 256, 50000
DELTA = 0.005
ROWS = T * K            # 2500
RP = 2560               # rows padded to 128 multiple
NB = RP // 128          # 20 row blocks
VP = 50176              # vocab padded; per-core 6272 = 12*512 + 128
VC = VP // N_CORES      # 6272 per-core vocab columns
KT = R // 128           # 2 contraction sub-tiles
CHUNK_SIZES = [1, 1] + [2] * 8 + [1, 1]  # blocks per chunk (sum = NB)
KLCH = 500              # KL column chunk


def _vtiles(vc):
    ts = [512] * (vc // 512)
    if vc % 512:
        ts.append(vc % 512)
    return ts


def _vgroups(tiles):
    """Group consecutive v-tiles into <=4-tile (one PSUM slot) groups."""
    groups, i = [], 0
    while i < len(tiles):
        j = min(i + 4, len(tiles))
        groups.append((i, j))
        i = j
    return groups


def _emit(nc, tc, io):
    wth, wtl, ath, atl, muT, lsT, nbias, out, kl = io
    tiles = _vtiles(VC)
    offs = np.concatenate([[0], np.cumsum(tiles)]).tolist()
    groups = _vgroups(tiles)

    with (
        tc.tile_pool(name="wpool", bufs=1) as wp,
        tc.tile_pool(name="apool", bufs=1) as apool,
        tc.tile_pool(name="misc", bufs=1) as misc,
        tc.tile_pool(name="klp", bufs=2) as klp,
        tc.tile_pool(name="upool", bufs=4) as up,
        tc.tile_pool(name="pspool", bufs=2, space="PSUM") as pp,
        tc.tile_pool(name="stats", bufs=3) as stp,
        tc.tile_pool(name="ccp", bufs=2, space="DRAM") as ccp,
    ):
        # resident weights / alphas (fp16 hi+lo, per contraction sub-tile).
        # Alphas first (small, needed by every matmul), then W in column
        # chunks so the first vocab group's matmuls start early.
        wh = []
        wl = []
        ah = []
        al = []
        for kt in range(KT):
            t = apool.tile([128, RP], F16, tag=f"ah{kt}")
            nc.sync.dma_start(t[:], ath[kt * 128:(kt + 1) * 128, :])
            ah.append(t)
            t = apool.tile([128, RP], F16, tag=f"al{kt}")
            nc.sync.dma_start(t[:], atl[kt * 128:(kt + 1) * 128, :])
            al.append(t)
            twh = wp.tile([128, VC], F16, tag=f"wh{kt}")
            wh.append(twh)
            twl = wp.tile([128, VC], F16, tag=f"wl{kt}")
            wl.append(twl)
        nb = misc.tile([128, NB], F32, tag="nbias")
        nc.sync.dma_start(nb[:], nbias[:])
        for v0 in range(0, VC, 2048):
            v1 = min(v0 + 2048, VC)
            for kt in range(KT):
                nc.sync.dma_start(wh[kt][:, v0:v1],
                                  wth[kt * 128:(kt + 1) * 128, v0:v1])
                nc.sync.dma_start(wl[kt][:, v0:v1],
                                  wtl[kt * 128:(kt + 1) * 128, v0:v1])

        # ---- KL helpers (emitted interleaved with main chunks so the
        # DVE/ACT work lands in idle slots mid-run, not in the tail) ----
        kvec = misc.tile([128, 1], F32, tag="kvec")
        nc.vector.memset(kvec[:], 0.0)
        c0s = np.float32(1.0) / (np.float32(1.0) + np.float32(1e-6))
        log_delta = np.float32(np.log(np.float32(DELTA)))
        c1s = np.float32(1.0) / (np.exp(log_delta, dtype=np.float32) + np.float32(1e-6))

        def emit_kl_chunk(kt, c0):
            c1 = min(c0 + KLCH, ROWS)
            w = c1 - c0
            mu_t = klp.tile([128, KLCH], F32, tag="klmu")
            nc.sync.dma_start(mu_t[:, :w], muT[kt * 128:(kt + 1) * 128, c0:c1])
            ls_t = klp.tile([128, KLCH], F32, tag="klls")
            nc.sync.dma_start(ls_t[:, :w], lsT[kt * 128:(kt + 1) * 128, c0:c1])
            e = klp.tile([128, KLCH], F32, tag="kle")
            nc.scalar.activation(e[:, :w], ls_t[:, :w], EXP)
            # d = mu - p_alpha  (p_alpha = alphas shifted K cols; 0 for t=0)
            d = klp.tile([128, KLCH], F32, tag="kld")
            if c0 < K:
                nc.vector.tensor_copy(d[:, :K], mu_t[:, :K])
                nc.vector.tensor_sub(d[:, K:w], mu_t[:, K:w], ah[kt][:, 0:c1 - K])
                nc.vector.tensor_sub(d[:, K:w], d[:, K:w], al[kt][:, 0:c1 - K])
            else:
                nc.vector.tensor_sub(d[:, :w], mu_t[:, :w], ah[kt][:, c0 - K:c1 - K])
                nc.vector.tensor_sub(d[:, :w], d[:, :w], al[kt][:, c0 - K:c1 - K])
            nc.vector.tensor_mul(d[:, :w], d[:, :w], d[:, :w])
            nc.vector.tensor_add(e[:, :w], e[:, :w], d[:, :w])
            if c0 < K:
                nc.vector.tensor_scalar_mul(e[:, :K], e[:, :K], float(c0s))
                nc.vector.tensor_scalar_mul(e[:, K:w], e[:, K:w], float(c1s))
            else:
                nc.vector.tensor_scalar_mul(e[:, :w], e[:, :w], float(c1s))
            ra = klp.tile([128, 1], F32, tag="klra")
            nc.vector.reduce_sum(ra[:], e[:, :w], axis=X)
            rb = klp.tile([128, 1], F32, tag="klrb")
            nc.vector.reduce_sum(rb[:], ls_t[:, :w], axis=X)
            nc.vector.tensor_add(kvec[:], kvec[:], ra[:])
            nc.vector.tensor_sub(kvec[:], kvec[:], rb[:])

        kl_jobs = [(kt, c0) for kt in range(KT) for c0 in range(0, ROWS, KLCH)]

        # warmup AllReduce: the first couple of collectives run 3x slower
        # (cc firmware cold); burn that during the W-load phase.
        wu_in = ccp.tile([128, 1], F32, tag="wuin")
        wu_out = ccp.tile([128, 1], F32, tag="wuout")
        nc.gpsimd.dma_start(wu_in[:], nb[:, 0:1])
        nc.gpsimd.collective_compute(
            "AllReduce",
            mybir.AluOpType.add,
            replica_groups=[list(range(N_CORES))],
            ins=[wu_in.opt()],
            outs=[wu_out.opt()],
        )

        # ---- main loop: matmul -> exp(+rowsum) -> allreduce -> scale -> out ----
        stationaries = []
        for kt in range(KT):
            stationaries.append((ah[kt], (wh[kt], wl[kt])))
            stationaries.append((al[kt], (wh[kt],)))
        n_acc = sum(len(m) for _, m in stationaries)  # 6 accumulating mms per slice

        b0 = 0
        for ch, csz in enumerate(CHUNK_SIZES):
            uts = []
            sch = stp.tile([128, csz], F32, tag="sch")
            for bl in range(csz):
                b = b0 + bl
                ut = up.tile([128, VC], F32, tag="u")
                uts.append(ut)
                sacc = stp.tile([128, len(groups)], F32, tag="sacc")
                for gi, (g0, g1) in enumerate(groups):
                    gw = offs[g1] - offs[g0]
                    pt = pp.tile([128, 2048], F32, tag="ps")
                    acc_i = 0
                    for sta, movs in stationaries:
                        for mov in movs:
                            for vt in range(g0, g1):
                                o = offs[vt] - offs[g0]
                                n = tiles[vt]
                                nc.tensor.matmul(
                                    pt[:, o:o + n],
                                    sta[:, b * 128:(b + 1) * 128],
                                    mov[:, offs[vt]:offs[vt] + n],
                                    start=(acc_i == 0),
                                    stop=(acc_i == n_acc - 1),
                                )
                            acc_i += 1
                    nc.scalar.activation(
                        ut[:, offs[g0]:offs[g0] + gw],
                        pt[:, :gw],
                        EXP,
                        bias=nb[:, b:b + 1],
                        scale=1.0,
                        accum_out=sacc[:, gi:gi + 1],
                    )
                nc.vector.reduce_sum(sch[:, bl:bl + 1], sacc[:], axis=X)
            ccin = ccp.tile([128, csz], F32, tag="ccin")
            ccout = ccp.tile([128, csz], F32, tag="ccout")
            nc.gpsimd.dma_start(ccin[:], sch[:])
            nc.gpsimd.collective_compute(
                "AllReduce",
                mybir.AluOpType.add,
                replica_groups=[list(range(N_CORES))],
                ins=[ccin.opt()],
                outs=[ccout.opt()],
            )
            sg = stp.tile([128, csz], F32, tag="sg")
            nc.gpsimd.dma_start(sg[:], ccout[:])
            rec = stp.tile([128, csz], F32, tag="rec")
            nc.vector.reciprocal(rec[:], sg[:])
            for bl in range(csz):
                b = b0 + bl
                nc.vector.tensor_scalar_mul(uts[bl][:], uts[bl][:], rec[:, bl:bl + 1])
                r0 = b * 128
                r1 = min(ROWS, r0 + 128)
                if r1 > r0:
                    nc.sync.dma_start(out[r0:r1, :], uts[bl][:r1 - r0, :])
            b0 += csz
            if kl_jobs:
                emit_kl_chunk(*kl_jobs.pop(0))
        while kl_jobs:
            emit_kl_chunk(*kl_jobs.pop(0))

        # cross-partition sum of the KL vector via DRAM bounce
        kd = ccp.tile([128, 1], F32, tag="kldram")
        nc.sync.dma_start(kd[:], kvec[:])
        krow = misc.tile([1, 128], F32, tag="krow")
        nc.sync.dma_start(krow[0:1, :], kd[:].rearrange("a b -> b a"))
        ks = misc.tile([1, 1], F32, tag="ks")
        nc.vector.reduce_sum(ks[0:1, :], krow[0:1, :], axis=X)
        nc.sync.dma_start(kl[:], ks[0:1, 0:1])



_CACHE = {}


def _build():
    if "nc" in _CACHE:
        return _CACHE["nc"], _CACHE["io_names"]
    nc = bacc.Bacc("TRN2", target_bir_lowering=False, debug=False,
                   num_devices=N_CORES)
    wth = nc.dram_tensor("wth", [R, VC], F16, kind="ExternalInput").ap()
    wtl = nc.dram_tensor("wtl", [R, VC], F16, kind="ExternalInput").ap()
    ath = nc.dram_tensor("ath", [R, RP], F16, kind="ExternalInput").ap()
    atl = nc.dram_tensor("atl", [R, RP], F16, kind="ExternalInput").ap()
    muT = nc.dram_tensor("muT", [R, ROWS], F32, kind="ExternalInput").ap()
    lsT = nc.dram_tensor("lsT", [R, ROWS], F32, kind="ExternalInput").ap()
    nbias = nc.dram_tensor("nbias", [128, NB], F32, kind="ExternalInput").ap()
    out = nc.dram_tensor("out", [ROWS, VC], F32, kind="ExternalOutput").ap()
    kl = nc.dram_tensor("kl", [1, 1], F32, kind="ExternalOutput").ap()
    with tile.TileContext(nc) as tc:
        _emit(nc, tc, (wth, wtl, ath, atl, muT, lsT, nbias, out, kl))
    nc.compile()
    _CACHE["nc"] = nc
    _CACHE["io_names"] = None
    return nc, None


def _bf16_split(x):
    hi = x.astype(np.float16)
    lo = (x - hi.astype(np.float32)).astype(np.float16)
    return np.ascontiguousarray(hi), np.ascontiguousarray(lo)


def kernel(mu_q_alpha, logsigma_q_alpha, eps, W):
    mu = np.asarray(mu_q_alpha, dtype=np.float32)
    ls = np.asarray(logsigma_q_alpha, dtype=np.float32)
    ep = np.asarray(eps, dtype=np.float32)
    Wf = np.asarray(W, dtype=np.float32)

    # host prep: alphas ([T,K,R] row-major -> rows t*K+k), transposes, splits
    mu2 = np.transpose(mu, (1, 0, 2)).reshape(ROWS, R)
    ls2 = np.transpose(ls, (1, 0, 2)).reshape(ROWS, R)
    ep2 = ep.reshape(ROWS, R)
    alphas = mu2 + ep2 * np.exp(0.5 * ls2)

    ap = np.zeros((RP, R), np.float32)
    ap[:ROWS] = alphas
    aT = np.ascontiguousarray(ap.T)              # [256, 2560]
    ath, atl = _bf16_split(aT)
    muT = np.ascontiguousarray(mu2.T)            # [256, 2500]
    lsT = np.ascontiguousarray(ls2.T)

    Wp = np.zeros((VP, R), np.float32)
    Wp[:V] = Wf
    wt_h = []
    wt_l = []
    for c in range(N_CORES):
        Wc = np.ascontiguousarray(Wp[c * VC:(c + 1) * VC].T)  # [256, 6400]
        h, lo = _bf16_split(Wc)
        wt_h.append(h)
        wt_l.append(lo)

    # sampled per-row upper-ish bound for the exp shift (any value within
    # ~(-87,+87) of the true row max keeps fp32 exact; verified below)
    samp = alphas @ Wf[::499].T                  # [2500, ~101]
    brow_full = np.zeros(RP, np.float32)
    brow_full[:ROWS] = samp.max(axis=1)

    nc, _ = _build()

    for _attempt in range(4):
        nbias = np.ascontiguousarray(
            -brow_full.reshape(NB, 128).T.astype(np.float32))  # [128, NB]
        in_maps = []
        for c in range(N_CORES):
            in_maps.append({
                "wth": wt_h[c], "wtl": wt_l[c],
                "ath": ath, "atl": atl,
                "muT": muT, "lsT": lsT,
                "nbias": nbias,
            })
        res = bass_utils.run_bass_kernel_spmd(
            nc, in_maps, core_ids=list(range(N_CORES)))
        beta2d = np.concatenate(
            [res.results[c]["out"] for c in range(N_CORES)], axis=1)[:, :V]
        # softmax rows must be finite and sum to ~1; a too-low bias overflows
        # exp (NaN row) or overflows the row-sum (all-zero row). Bump & retry.
        bad = (~np.isfinite(beta2d).all(axis=1)) | (
            np.abs(beta2d.sum(axis=1, dtype=np.float64) - 1.0) > 1e-3)
        if not bad.any():
            break
        brow_full[:ROWS][bad] += 60.0  # never hit for randn-scale inputs
    beta = beta2d.reshape(T, K, V)

    log_delta = np.float32(np.log(np.float32(DELTA)))
    c_const = float(R) * (K * (-1.0) + (ROWS - K) * (-1.0 + float(log_delta)))
    kl_alpha = np.float32(0.5 * (float(res.results[0]["kl"][0, 0]) + c_const))
    return beta, kl_alpha


if __name__ == "__main__":
    import reference

    inputs = {k: np.asarray(v) for k, v in reference.setup_inputs().items()}
    beta, kl_alpha = kernel(**inputs)
    print(beta.shape, beta.dtype, kl_alpha)


# revision 10
# speedup vs baseline: 1.1258x; 1.1258x over previous
"""Trainium2 Bass kernel for nn_Decoder: beta = softmax(alphas @ W^T), kl_alpha.

Strategy (8 NeuronCores, vocab-sharded tensor parallel):
  - Host prep: alphas = mu + eps*exp(0.5*ls) (tiny, [2500,256]); transpose to
    [256, 2500] and pad rows to 2560; split into fp16 hi/lo pair. W is padded
    to 50176 vocab, transposed, column-sharded 8x6272, fp16 hi/lo split.
  - Device, per core: logits tile = ah@wh + al@wh + ah@wl (3-term fp16
    compensated product ~ fp32 precision, full fp16 matmul rate) into PSUM;
    ScalarE computes u = exp(logits + bias_row) PSUM->SBUF with fused
    per-row accumulation (row-sums). Per 256-row chunk, the [128,2] row-sums
    are AllReduced (add) across the 8 cores; VectorE multiplies the SBUF-
    resident u in place by 1/S and DMAs straight to the output shard. The
    u tiles never leave SBUF unnormalized, so HBM traffic is one output write.
  - bias_row = -(sampled row max) from a strided host-side sample; any
    row bias within ~(-87,+87) of the true row max is exact (fp32 range),
    and a host-side finite check retries with a bumped bias if ever violated.
  - KL is computed on-device from muT/lsT/alphasT (elementwise + reductions),
    chunked so it fills VectorE idle slots; the data-independent constant
    part of the KL sum is added back on the host.
"""

import numpy as np
import ml_dtypes

import concourse.bacc as bacc
import concourse.mybir as mybir
import concourse.tile as tile
from concourse import bass_utils

F32 = mybir.dt.float32
F16 = mybir.dt.float16
X = mybir.AxisListType.X
EXP = mybir.ActivationFunctionType.Exp

N_CORES = 8
T, K, R, V = 50, 50, 256, 50000
DELTA = 0.005
ROWS = T * K            # 2500
RP = 2560               # rows padded to 128 multiple
NB = RP // 128          # 20 row blocks
VP = 50176              # vocab padded; per-core 6272 = 12*512 + 128
VC = VP // N_CORES      # 6272 per-core vocab columns
KT = R // 128           # 2 contraction sub-tiles
CHUNK_SIZES = [2] * 10  # blocks per chunk (sum = NB)
KLCH = 500              # KL column chunk


def _vtiles(vc):
    ts = [512] * (vc // 512)
    if vc % 512:
        ts.append(vc % 512)
    return ts


def _vgroups(tiles):
    """Group consecutive v-tiles into <=4-tile (one PSUM slot) groups."""
    groups, i = [], 0
    while i < len(tiles):
        j = min(i + 4, len(tiles))
        groups.append((i, j))
        i = j
    return groups


def _emit(nc, tc, io):
    wth, wtl, ath, atl, muT, lsT, nbias, out, kl = io
    tiles = _vtiles(VC)
    offs = np.concatenate([[0], np.cumsum(tiles)]).tolist()
    groups = _vgroups(tiles)

    with (
        tc.tile_pool(name="wpool", bufs=1) as wp,
        tc.tile_pool(name="apool", bufs=1) as apool,
        tc.tile_pool(name="misc", bufs=1) as misc,
        tc.tile_pool(name="klp", bufs=2) as klp,
        tc.tile_pool(name="upool", bufs=4) as up,
        tc.tile_pool(name="pspool", bufs=2, space="PSUM") as pp,
        tc.tile_pool(name="stats", bufs=3) as stp,
        tc.tile_pool(name="ccp", bufs=2, space="DRAM") as ccp,
    ):
        # resident weights / alphas (fp16 hi+lo, per contraction sub-tile).
        # Alphas first (small, needed by every matmul), then W in column
        # chunks so the first vocab group's matmuls start early.
        wh = []
        wl = []
        ah = []
        al = []
        for kt in range(KT):
            t = apool.tile([128, RP], F16, tag=f"ah{kt}")
            nc.sync.dma_start(t[:], ath[kt * 128:(kt + 1) * 128, :])
            ah.append(t)
            t = apool.tile([128, RP], F16, tag=f"al{kt}")
            nc.sync.dma_start(t[:], atl[kt * 128:(kt + 1) * 128, :])
            al.append(t)
            twh = wp.tile([128, VC], F16, tag=f"wh{kt}")
            wh.append(twh)
            twl = wp.tile([128, VC], F16, tag=f"wl{kt}")
            wl.append(twl)
        nb = misc.tile([128, NB], F32, tag="nbias")
        nc.sync.dma_start(nb[:], nbias[:])
        for v0 in range(0, VC, 2048):
            v1 = min(v0 + 2048, VC)
            for kt in range(KT):
                nc.sync.dma_start(wh[kt][:, v0:v1],
                                  wth[kt * 128:(kt + 1) * 128, v0:v1])
                nc.sync.dma_start(wl[kt][:, v0:v1],
                                  wtl[kt * 128:(kt + 1) * 128, v0:v1])

        # ---- KL helpers (emitted interleaved with main chunks so the
        # DVE/ACT work lands in idle slots mid-run, not in the tail) ----
        kvec = misc.tile([128, 1], F32, tag="kvec")
        nc.vector.memset(kvec[:], 0.0)
        c0s = np.float32(1.0) / (np.float32(1.0) + np.float32(1e-6))
        log_delta = np.float32(np.log(np.float32(DELTA)))
        c1s = np.float32(1.0) / (np.exp(log_delta, dtype=np.float32) + np.float32(1e-6))

        def emit_kl_chunk(kt, c0):
            c1 = min(c0 + KLCH, ROWS)
            w = c1 - c0
            mu_t = klp.tile([128, KLCH], F32, tag="klmu")
            nc.sync.dma_start(mu_t[:, :w], muT[kt * 128:(kt + 1) * 128, c0:c1])
            ls_t = klp.tile([128, KLCH], F32, tag="klls")
            nc.sync.dma_start(ls_t[:, :w], lsT[kt * 128:(kt + 1) * 128, c0:c1])
            e = klp.tile([128, KLCH], F32, tag="kle")
            nc.scalar.activation(e[:, :w], ls_t[:, :w], EXP)
            # d = mu - p_alpha  (p_alpha = alphas shifted K cols; 0 for t=0)
            d = klp.tile([128, KLCH], F32, tag="kld")
            if c0 < K:
                nc.vector.tensor_copy(d[:, :K], mu_t[:, :K])
                nc.vector.tensor_sub(d[:, K:w], mu_t[:, K:w], ah[kt][:, 0:c1 - K])
                nc.vector.tensor_sub(d[:, K:w], d[:, K:w], al[kt][:, 0:c1 - K])
            else:
                nc.vector.tensor_sub(d[:, :w], mu_t[:, :w], ah[kt][:, c0 - K:c1 - K])
                nc.vector.tensor_sub(d[:, :w], d[:, :w], al[kt][:, c0 - K:c1 - K])
            nc.vector.tensor_mul(d[:, :w], d[:, :w], d[:, :w])
            nc.vector.tensor_add(e[:, :w], e[:, :w], d[:, :w])
            if c0 < K:
                nc.vector.tensor_scalar_mul(e[:, :K], e[:, :K], float(c0s))
                nc.vector.tensor_scalar_mul(e[:, K:w], e[:, K:w], float(c1s))
            else:
                nc.vector.tensor_scalar_mul(e[:, :w], e[:, :w], float(c1s))
            ra = klp.tile([128, 1], F32, tag="klra")
            nc.vector.reduce_sum(ra[:], e[:, :w], axis=X)
            rb = klp.tile([128, 1], F32, tag="klrb")
            nc.vector.reduce_sum(rb[:], ls_t[:, :w], axis=X)
            nc.vector.tensor_add(kvec[:], kvec[:], ra[:])
            nc.vector.tensor_sub(kvec[:], kvec[:], rb[:])

        kl_jobs = [(kt, c0) for kt in range(KT) for c0 in range(0, ROWS, KLCH)]

        # ---- main loop: matmul -> exp(+rowsum) -> allreduce -> scale -> out ----
        stationaries = []
        for kt in range(KT):
            stationaries.append((ah[kt], (wh[kt], wl[kt])))
            stationaries.append((al[kt], (wh[kt],)))
        n_acc = sum(len(m) for _, m in stationaries)  # 6 accumulating mms per slice

        b0 = 0
        for ch, csz in enumerate(CHUNK_SIZES):
            uts = []
            sch = stp.tile([128, csz], F32, tag="sch")
            for bl in range(csz):
                b = b0 + bl
                ut = up.tile([128, VC], F32, tag="u")
                uts.append(ut)
                sacc = stp.tile([128, len(groups)], F32, tag="sacc")
                for gi, (g0, g1) in enumerate(groups):
                    gw = offs[g1] - offs[g0]
                    pt = pp.tile([128, 2048], F32, tag="ps")
                    acc_i = 0
                    for sta, movs in stationaries:
                        for mov in movs:
                            for vt in range(g0, g1):
                                o = offs[vt] - offs[g0]
                                n = tiles[vt]
                                nc.tensor.matmul(
                                    pt[:, o:o + n],
                                    sta[:, b * 128:(b + 1) * 128],
                                    mov[:, offs[vt]:offs[vt] + n],
                                    start=(acc_i == 0),
                                    stop=(acc_i == n_acc - 1),
                                )
                            acc_i += 1
                    nc.scalar.activation(
                        ut[:, offs[g0]:offs[g0] + gw],
                        pt[:, :gw],
                        EXP,
                        bias=nb[:, b:b + 1],
                        scale=1.0,
                        accum_out=sacc[:, gi:gi + 1],
                    )
                nc.vector.reduce_sum(sch[:, bl:bl + 1], sacc[:], axis=X)
            ccin = ccp.tile([128, csz], F32, tag="ccin")
            ccout = ccp.tile([128, csz], F32, tag="ccout")
            nc.gpsimd.dma_start(ccin[:], sch[:])
            nc.gpsimd.collective_compute(
                "AllReduce",
                mybir.AluOpType.add,
                replica_groups=[list(range(N_CORES))],
                ins=[ccin.opt()],
                outs=[ccout.opt()],
            )
            sg = stp.tile([128, csz], F32, tag="sg")
            nc.gpsimd.dma_start(sg[:], ccout[:])
            rec = stp.tile([128, csz], F32, tag="rec")
            nc.vector.reciprocal(rec[:], sg[:])
            for bl in range(csz):
                b = b0 + bl
                nc.vector.tensor_scalar_mul(uts[bl][:], uts[bl][:], rec[:, bl:bl + 1])
                r0 = b * 128
                r1 = min(ROWS, r0 + 128)
                if r1 > r0:
                    nc.sync.dma_start(out[r0:r1, :], uts[bl][:r1 - r0, :])
            b0 += csz
            # keep KL work out of the busy head/tail windows
            if ch >= 2:
                for _ in range(2):
                    if kl_jobs:
                        emit_kl_chunk(*kl_jobs.pop(0))
        while kl_jobs:
            emit_kl_chunk(*kl_jobs.pop(0))

        # cross-partition sum of the KL vector via DRAM bounce
        kd = ccp.tile([128, 1], F32, tag="kldram")
        nc.sync.dma_start(kd[:], kvec[:])
        krow = misc.tile([1, 128], F32, tag="krow")
        nc.sync.dma_start(krow[0:1, :], kd[:].rearrange("a b -> b a"))
        ks = misc.tile([1, 1], F32, tag="ks")
        nc.vector.reduce_sum(ks[0:1, :], krow[0:1, :], axis=X)
        nc.sync.dma_start(kl[:], ks[0:1, 0:1])



_CACHE = {}


def _build():
    if "nc" in _CACHE:
        return _CACHE["nc"], _CACHE["io_names"]
    nc = bacc.Bacc("TRN2", target_bir_lowering=False, debug=False,
                   num_devices=N_CORES)
    wth = nc.dram_tensor("wth", [R, VC], F16, kind="ExternalInput").ap()
    wtl = nc.dram_tensor("wtl", [R, VC], F16, kind="ExternalInput").ap()
    ath = nc.dram_tensor("ath", [R, RP], F16, kind="ExternalInput").ap()
    atl = nc.dram_tensor("atl", [R, RP], F16, kind="ExternalInput").ap()
    muT = nc.dram_tensor("muT", [R, ROWS], F32, kind="ExternalInput").ap()
    lsT = nc.dram_tensor("lsT", [R, ROWS], F32, kind="ExternalInput").ap()
    nbias = nc.dram_tensor("nbias", [128, NB], F32, kind="ExternalInput").ap()
    out = nc.dram_tensor("out", [ROWS, VC], F32, kind="ExternalOutput").ap()
    kl = nc.dram_tensor("kl", [1, 1], F32, kind="ExternalOutput").ap()
    with tile.TileContext(nc) as tc:
        _emit(nc, tc, (wth, wtl, ath, atl, muT, lsT, nbias, out, kl))
    nc.compile()
    _CACHE["nc"] = nc
    _CACHE["io_names"] = None
    return nc, None


def _bf16_split(x):
    hi = x.astype(np.float16)
    lo = (x - hi.astype(np.float32)).astype(np.float16)
    return np.ascontiguousarray(hi), np.ascontiguousarray(lo)


def kernel(mu_q_alpha, logsigma_q_alpha, eps, W):
    mu = np.asarray(mu_q_alpha, dtype=np.float32)
    ls = np.asarray(logsigma_q_alpha, dtype=np.float32)
    ep = np.asarray(eps, dtype=np.float32)
    Wf = np.asarray(W, dtype=np.float32)

    # host prep: alphas ([T,K,R] row-major -> rows t*K+k), transposes, splits
    mu2 = np.transpose(mu, (1, 0, 2)).reshape(ROWS, R)
    ls2 = np.transpose(ls, (1, 0, 2)).reshape(ROWS, R)
    ep2 = ep.reshape(ROWS, R)
    alphas = mu2 + ep2 * np.exp(0.5 * ls2)

    ap = np.zeros((RP, R), np.float32)
    ap[:ROWS] = alphas
    aT = np.ascontiguousarray(ap.T)              # [256, 2560]
    ath, atl = _bf16_split(aT)
    muT = np.ascontiguousarray(mu2.T)            # [256, 2500]
    lsT = np.ascontiguousarray(ls2.T)

    Wp = np.zeros((VP, R), np.float32)
    Wp[:V] = Wf
    wt_h = []
    wt_l = []
    for c in range(N_CORES):
        Wc = np.ascontiguousarray(Wp[c * VC:(c + 1) * VC].T)  # [256, 6400]
        h, lo = _bf16_split(Wc)
        wt_h.append(h)
        wt_l.append(lo)

    # sampled per-row upper-ish bound for the exp shift (any value within
    # ~(-87,+87) of the true row max keeps fp32 exact; verified below)
    samp = alphas @ Wf[::499].T                  # [2500, ~101]
    brow_full = np.zeros(RP, np.float32)
    brow_full[:ROWS] = samp.max(axis=1)

    nc, _ = _build()

    for _attempt in range(4):
        nbias = np.ascontiguousarray(
            -brow_full.reshape(NB, 128).T.astype(np.float32))  # [128, NB]
        in_maps = []
        for c in range(N_CORES):
            in_maps.append({
                "wth": wt_h[c], "wtl": wt_l[c],
                "ath": ath, "atl": atl,
                "muT": muT, "lsT": lsT,
                "nbias": nbias,
            })
        res = bass_utils.run_bass_kernel_spmd(
            nc, in_maps, core_ids=list(range(N_CORES)))
        beta2d = np.concatenate(
            [res.results[c]["out"] for c in range(N_CORES)], axis=1)[:, :V]
        # softmax rows must be finite and sum to ~1; a too-low bias overflows
        # exp (NaN row) or overflows the row-sum (all-zero row). Bump & retry.
        bad = (~np.isfinite(beta2d).all(axis=1)) | (
            np.abs(beta2d.sum(axis=1, dtype=np.float64) - 1.0) > 1e-3)
        if not bad.any():
            break
        brow_full[:ROWS][bad] += 60.0  # never hit for randn-scale inputs
    beta = beta2d.reshape(T, K, V)

    log_delta = np.float32(np.log(np.float32(DELTA)))
    c_const = float(R) * (K * (-1.0) + (ROWS - K) * (-1.0 + float(log_delta)))
    kl_alpha = np.float32(0.5 * (float(res.results[0]["kl"][0, 0]) + c_const))
    return beta, kl_alpha


if __name__ == "__main__":
    import reference

    inputs = {k: np.asarray(v) for k, v in reference.setup_inputs().items()}
    beta, kl_alpha = kernel(**inputs)
    print(beta.shape, beta.dtype, kl_alpha)


# revision 12
# speedup vs baseline: 1.1809x; 1.0490x over previous
"""Trainium2 Bass kernel for nn_Decoder: beta = softmax(alphas @ W^T), kl_alpha.

Strategy (8 NeuronCores, vocab-sharded tensor parallel):
  - Host prep: alphas = mu + eps*exp(0.5*ls) (tiny, [2500,256]); transpose to
    [256, 2500] and pad rows to 2560; split into fp16 hi/lo pair. W is padded
    to 50176 vocab, transposed, column-sharded 8x6272, fp16 hi/lo split.
  - Device, per core: logits tile = ah@wh + al@wh + ah@wl (3-term fp16
    compensated product ~ fp32 precision, full fp16 matmul rate) into PSUM;
    ScalarE computes u = exp(logits + bias_row) PSUM->SBUF with fused
    per-row accumulation (row-sums). Per 256-row chunk, the [128,2] row-sums
    are AllReduced (add) across the 8 cores; VectorE multiplies the SBUF-
    resident u in place by 1/S and DMAs straight to the output shard. The
    u tiles never leave SBUF unnormalized, so HBM traffic is one output write.
  - bias_row = -(sampled row max) from a strided host-side sample; any
    row bias within ~(-87,+87) of the true row max is exact (fp32 range),
    and a host-side finite check retries with a bumped bias if ever violated.
  - KL is computed on-device from muT/lsT/alphasT (elementwise + reductions),
    chunked so it fills VectorE idle slots; the data-independent constant
    part of the KL sum is added back on the host.
"""

import numpy as np
import ml_dtypes

import concourse.bacc as bacc
import concourse.mybir as mybir
import concourse.tile as tile
from concourse import bass_utils

F32 = mybir.dt.float32
F16 = mybir.dt.float16
X = mybir.AxisListType.X
EXP = mybir.ActivationFunctionType.Exp

N_CORES = 8
T, K, R, V = 50, 50, 256, 50000
DELTA = 0.005
ROWS = T * K            # 2500
RP = 2560               # rows padded to 128 multiple
NB = RP // 128          # 20 row blocks
VP = 50176              # vocab padded; per-core 6272 = 12*512 + 128
VC = VP // N_CORES      # 6272 per-core vocab columns
KT = R // 128           # 2 contraction sub-tiles
CHUNK_SIZES = [2] * 10  # blocks per chunk (sum = NB)
KLCH = 500              # KL column chunk


def _vtiles(vc):
    ts = [512] * (vc // 512)
    if vc % 512:
        ts.append(vc % 512)
    return ts


def _vgroups(tiles):
    """Group consecutive v-tiles into <=4-tile (one PSUM slot) groups."""
    groups, i = [], 0
    while i < len(tiles):
        j = min(i + 4, len(tiles))
        groups.append((i, j))
        i = j
    return groups


def _emit(nc, tc, io):
    wth, wtl, ath, atl, muT, lsT, nbias, out, kl = io
    tiles = _vtiles(VC)
    offs = np.concatenate([[0], np.cumsum(tiles)]).tolist()
    groups = _vgroups(tiles)

    with (
        tc.tile_pool(name="wpool", bufs=1) as wp,
        tc.tile_pool(name="apool", bufs=1) as apool,
        tc.tile_pool(name="misc", bufs=1) as misc,
        tc.tile_pool(name="klp", bufs=2) as klp,
        tc.tile_pool(name="upool", bufs=4) as up,
        tc.tile_pool(name="pspool", bufs=2, space="PSUM") as pp,
        tc.tile_pool(name="stats", bufs=3) as stp,
        tc.tile_pool(name="ccp", bufs=2, space="DRAM") as ccp,
    ):
        # resident weights / alphas (fp16 hi+lo, per contraction sub-tile).
        # Alphas first (small, needed by every matmul), then W in column
        # chunks so the first vocab group's matmuls start early.
        wh = []
        wl = []
        ah = []
        al = []
        for kt in range(KT):
            t = apool.tile([128, RP], F16, tag=f"ah{kt}")
            nc.sync.dma_start(t[:], ath[kt * 128:(kt + 1) * 128, :])
            ah.append(t)
            t = apool.tile([128, RP], F16, tag=f"al{kt}")
            nc.sync.dma_start(t[:], atl[kt * 128:(kt + 1) * 128, :])
            al.append(t)
            twh = wp.tile([128, VC], F16, tag=f"wh{kt}")
            wh.append(twh)
            twl = wp.tile([128, VC], F16, tag=f"wl{kt}")
            wl.append(twl)
        nb = misc.tile([128, NB], F32, tag="nbias")
        nc.sync.dma_start(nb[:], nbias[:])
        for v0 in range(0, VC, 2048):
            v1 = min(v0 + 2048, VC)
            for kt in range(KT):
                nc.sync.dma_start(wh[kt][:, v0:v1],
                                  wth[kt * 128:(kt + 1) * 128, v0:v1])
                nc.sync.dma_start(wl[kt][:, v0:v1],
                                  wtl[kt * 128:(kt + 1) * 128, v0:v1])

        # ---- KL helpers (emitted interleaved with main chunks so the
        # DVE/ACT work lands in idle slots mid-run, not in the tail) ----
        kvec = misc.tile([128, 1], F32, tag="kvec")
        nc.vector.memset(kvec[:], 0.0)
        c0s = np.float32(1.0) / (np.float32(1.0) + np.float32(1e-6))
        log_delta = np.float32(np.log(np.float32(DELTA)))
        c1s = np.float32(1.0) / (np.exp(log_delta, dtype=np.float32) + np.float32(1e-6))

        def emit_kl_chunk(kt, c0):
            c1 = min(c0 + KLCH, ROWS)
            w = c1 - c0
            mu_t = klp.tile([128, KLCH], F32, tag="klmu")
            nc.sync.dma_start(mu_t[:, :w], muT[kt * 128:(kt + 1) * 128, c0:c1])
            ls_t = klp.tile([128, KLCH], F32, tag="klls")
            nc.sync.dma_start(ls_t[:, :w], lsT[kt * 128:(kt + 1) * 128, c0:c1])
            e = klp.tile([128, KLCH], F32, tag="kle")
            nc.scalar.activation(e[:, :w], ls_t[:, :w], EXP)
            # d = mu - p_alpha  (p_alpha = alphas shifted K cols; 0 for t=0)
            d = klp.tile([128, KLCH], F32, tag="kld")
            if c0 < K:
                nc.vector.tensor_copy(d[:, :K], mu_t[:, :K])
                nc.vector.tensor_sub(d[:, K:w], mu_t[:, K:w], ah[kt][:, 0:c1 - K])
                nc.vector.tensor_sub(d[:, K:w], d[:, K:w], al[kt][:, 0:c1 - K])
            else:
                nc.vector.tensor_sub(d[:, :w], mu_t[:, :w], ah[kt][:, c0 - K:c1 - K])
                nc.vector.tensor_sub(d[:, :w], d[:, :w], al[kt][:, c0 - K:c1 - K])
            nc.vector.tensor_mul(d[:, :w], d[:, :w], d[:, :w])
            nc.vector.tensor_add(e[:, :w], e[:, :w], d[:, :w])
            if c0 < K:
                nc.vector.tensor_scalar_mul(e[:, :K], e[:, :K], float(c0s))
                nc.vector.tensor_scalar_mul(e[:, K:w], e[:, K:w], float(c1s))
            else:
                nc.vector.tensor_scalar_mul(e[:, :w], e[:, :w], float(c1s))
            ra = klp.tile([128, 1], F32, tag="klra")
            nc.vector.reduce_sum(ra[:], e[:, :w], axis=X)
            rb = klp.tile([128, 1], F32, tag="klrb")
            nc.vector.reduce_sum(rb[:], ls_t[:, :w], axis=X)
            nc.vector.tensor_add(kvec[:], kvec[:], ra[:])
            nc.vector.tensor_sub(kvec[:], kvec[:], rb[:])

        kl_jobs = [(kt, c0) for kt in range(KT) for c0 in range(0, ROWS, KLCH)]

        # ---- main loop: matmul -> exp(+rowsum) -> allreduce -> scale -> out ----
        stationaries = []
        for kt in range(KT):
            stationaries.append((ah[kt], (wh[kt], wl[kt])))
            stationaries.append((al[kt], (wh[kt],)))
        n_acc = sum(len(m) for _, m in stationaries)  # 6 accumulating mms per slice

        # u is staged per block as two column-half tiles so slots recycle at
        # half-block granularity and scale/store pipeline per half.
        HSPLIT = (len(tiles) + 1) // 2  # v-tile index of the half boundary
        hoff = offs[HSPLIT]
        halves = [(0, HSPLIT, 0, hoff, "ua"),
                  (HSPLIT, len(tiles), hoff, VC - hoff, "ub")]
        hgroups = {}
        for h0, h1, hbase, hw, htag in halves:
            gs, i = [], h0
            while i < h1:
                j = min(i + 4, h1)
                gs.append((i, j))
                i = j
            hgroups[htag] = gs
        n_groups = sum(len(g) for g in hgroups.values())

        b0 = 0
        for ch, csz in enumerate(CHUNK_SIZES):
            uts = []
            sch = stp.tile([128, csz], F32, tag="sch")
            for bl in range(csz):
                b = b0 + bl
                blk_halves = {}
                sacc = stp.tile([128, n_groups], F32, tag="sacc")
                gi = 0
                for h0, h1, hbase, hw, htag in halves:
                    ut = up.tile([128, hw], F32, tag=htag)
                    blk_halves[htag] = ut
                    for g0, g1 in hgroups[htag]:
                        gw = offs[g1] - offs[g0]
                        pt = pp.tile([128, 2048], F32, tag="ps")
                        acc_i = 0
                        for sta, movs in stationaries:
                            for mov in movs:
                                for vt in range(g0, g1):
                                    o = offs[vt] - offs[g0]
                                    n = tiles[vt]
                                    nc.tensor.matmul(
                                        pt[:, o:o + n],
                                        sta[:, b * 128:(b + 1) * 128],
                                        mov[:, offs[vt]:offs[vt] + n],
                                        start=(acc_i == 0),
                                        stop=(acc_i == n_acc - 1),
                                    )
                                acc_i += 1
                        nc.scalar.activation(
                            ut[:, offs[g0] - hbase:offs[g0] - hbase + gw],
                            pt[:, :gw],
                            EXP,
                            bias=nb[:, b:b + 1],
                            scale=1.0,
                            accum_out=sacc[:, gi:gi + 1],
                        )
                        gi += 1
                uts.append(blk_halves)
                nc.vector.reduce_sum(sch[:, bl:bl + 1], sacc[:], axis=X)
            ccin = ccp.tile([128, csz], F32, tag="ccin")
            ccout = ccp.tile([128, csz], F32, tag="ccout")
            nc.gpsimd.dma_start(ccin[:], sch[:])
            nc.gpsimd.collective_compute(
                "AllReduce",
                mybir.AluOpType.add,
                replica_groups=[list(range(N_CORES))],
                ins=[ccin.opt()],
                outs=[ccout.opt()],
            )
            sg = stp.tile([128, csz], F32, tag="sg")
            nc.gpsimd.dma_start(sg[:], ccout[:])
            rec = stp.tile([128, csz], F32, tag="rec")
            nc.vector.reciprocal(rec[:], sg[:])
            for bl in range(csz):
                b = b0 + bl
                r0 = b * 128
                r1 = min(ROWS, r0 + 128)
                for h0, h1, hbase, hw, htag in halves:
                    ut = uts[bl][htag]
                    nc.vector.tensor_scalar_mul(ut[:], ut[:], rec[:, bl:bl + 1])
                    if r1 > r0:
                        nc.sync.dma_start(out[r0:r1, hbase:hbase + hw],
                                          ut[:r1 - r0, :])
            b0 += csz
            # keep KL work out of the busy head/tail windows
            if ch >= 2:
                for _ in range(2):
                    if kl_jobs:
                        emit_kl_chunk(*kl_jobs.pop(0))
        while kl_jobs:
            emit_kl_chunk(*kl_jobs.pop(0))

        # cross-partition sum of the KL vector via DRAM bounce
        kd = ccp.tile([128, 1], F32, tag="kldram")
        nc.sync.dma_start(kd[:], kvec[:])
        krow = misc.tile([1, 128], F32, tag="krow")
        nc.sync.dma_start(krow[0:1, :], kd[:].rearrange("a b -> b a"))
        ks = misc.tile([1, 1], F32, tag="ks")
        nc.vector.reduce_sum(ks[0:1, :], krow[0:1, :], axis=X)
        nc.sync.dma_start(kl[:], ks[0:1, 0:1])



_CACHE = {}


def _build():
    if "nc" in _CACHE:
        return _CACHE["nc"], _CACHE["io_names"]
    nc = bacc.Bacc("TRN2", target_bir_lowering=False, debug=False,
                   num_devices=N_CORES)
    wth = nc.dram_tensor("wth", [R, VC], F16, kind="ExternalInput").ap()
    wtl = nc.dram_tensor("wtl", [R, VC], F16, kind="ExternalInput").ap()
    ath = nc.dram_tensor("ath", [R, RP], F16, kind="ExternalInput").ap()
    atl = nc.dram_tensor("atl", [R, RP], F16, kind="ExternalInput").ap()
    muT = nc.dram_tensor("muT", [R, ROWS], F32, kind="ExternalInput").ap()
    lsT = nc.dram_tensor("lsT", [R, ROWS], F32, kind="ExternalInput").ap()
    nbias = nc.dram_tensor("nbias", [128, NB], F32, kind="ExternalInput").ap()
    out = nc.dram_tensor("out", [ROWS, VC], F32, kind="ExternalOutput").ap()
    kl = nc.dram_tensor("kl", [1, 1], F32, kind="ExternalOutput").ap()
    with tile.TileContext(nc) as tc:
        _emit(nc, tc, (wth, wtl, ath, atl, muT, lsT, nbias, out, kl))
    nc.compile()
    _CACHE["nc"] = nc
    _CACHE["io_names"] = None
    return nc, None


def _bf16_split(x):
    hi = x.astype(np.float16)
    lo = (x - hi.astype(np.float32)).astype(np.float16)
    return np.ascontiguousarray(hi), np.ascontiguousarray(lo)


def kernel(mu_q_alpha, logsigma_q_alpha, eps, W):
    mu = np.asarray(mu_q_alpha, dtype=np.float32)
    ls = np.asarray(logsigma_q_alpha, dtype=np.float32)
    ep = np.asarray(eps, dtype=np.float32)
    Wf = np.asarray(W, dtype=np.float32)

    # host prep: alphas ([T,K,R] row-major -> rows t*K+k), transposes, splits
    mu2 = np.transpose(mu, (1, 0, 2)).reshape(ROWS, R)
    ls2 = np.transpose(ls, (1, 0, 2)).reshape(ROWS, R)
    ep2 = ep.reshape(ROWS, R)
    alphas = mu2 + ep2 * np.exp(0.5 * ls2)

    ap = np.zeros((RP, R), np.float32)
    ap[:ROWS] = alphas
    aT = np.ascontiguousarray(ap.T)              # [256, 2560]
    ath, atl = _bf16_split(aT)
    muT = np.ascontiguousarray(mu2.T)            # [256, 2500]
    lsT = np.ascontiguousarray(ls2.T)

    Wp = np.zeros((VP, R), np.float32)
    Wp[:V] = Wf
    wt_h = []
    wt_l = []
    for c in range(N_CORES):
        Wc = np.ascontiguousarray(Wp[c * VC:(c + 1) * VC].T)  # [256, 6400]
        h, lo = _bf16_split(Wc)
        wt_h.append(h)
        wt_l.append(lo)

    # sampled per-row upper-ish bound for the exp shift (any value within
    # ~(-87,+87) of the true row max keeps fp32 exact; verified below)
    samp = alphas @ Wf[::499].T                  # [2500, ~101]
    brow_full = np.zeros(RP, np.float32)
    brow_full[:ROWS] = samp.max(axis=1)

    nc, _ = _build()

    for _attempt in range(4):
        nbias = np.ascontiguousarray(
            -brow_full.reshape(NB, 128).T.astype(np.float32))  # [128, NB]
        in_maps = []
        for c in range(N_CORES):
            in_maps.append({
                "wth": wt_h[c], "wtl": wt_l[c],
                "ath": ath, "atl": atl,
                "muT": muT, "lsT": lsT,
                "nbias": nbias,
            })
        res = bass_utils.run_bass_kernel_spmd(
            nc, in_maps, core_ids=list(range(N_CORES)))
        beta2d = np.concatenate(
            [res.results[c]["out"] for c in range(N_CORES)], axis=1)[:, :V]
        # softmax rows must be finite and sum to ~1; a too-low bias overflows
        # exp (NaN row) or overflows the row-sum (all-zero row). Bump & retry.
        bad = (~np.isfinite(beta2d).all(axis=1)) | (
            np.abs(beta2d.sum(axis=1, dtype=np.float64) - 1.0) > 1e-3)
        if not bad.any():
            break
        brow_full[:ROWS][bad] += 60.0  # never hit for randn-scale inputs
    beta = beta2d.reshape(T, K, V)

    log_delta = np.float32(np.log(np.float32(DELTA)))
    c_const = float(R) * (K * (-1.0) + (ROWS - K) * (-1.0 + float(log_delta)))
    kl_alpha = np.float32(0.5 * (float(res.results[0]["kl"][0, 0]) + c_const))
    return beta, kl_alpha


if __name__ == "__main__":
    import reference

    inputs = {k: np.asarray(v) for k, v in reference.setup_inputs().items()}
    beta, kl_alpha = kernel(**inputs)
    print(beta.shape, beta.dtype, kl_alpha)


# revision 13
# speedup vs baseline: 1.1822x; 1.0010x over previous
"""Trainium2 Bass kernel for nn_Decoder: beta = softmax(alphas @ W^T), kl_alpha.

Strategy (8 NeuronCores, vocab-sharded tensor parallel):
  - Host prep: alphas = mu + eps*exp(0.5*ls) (tiny, [2500,256]); transpose to
    [256, 2500] and pad rows to 2560; split into fp16 hi/lo pair. W is padded
    to 50176 vocab, transposed, column-sharded 8x6272, fp16 hi/lo split.
  - Device, per core: logits tile = ah@wh + al@wh + ah@wl (3-term fp16
    compensated product ~ fp32 precision, full fp16 matmul rate) into PSUM;
    ScalarE computes u = exp(logits + bias_row) PSUM->SBUF with fused
    per-row accumulation (row-sums). Per 256-row chunk, the [128,2] row-sums
    are AllReduced (add) across the 8 cores; VectorE multiplies the SBUF-
    resident u in place by 1/S and DMAs straight to the output shard. The
    u tiles never leave SBUF unnormalized, so HBM traffic is one output write.
  - bias_row = -(sampled row max) from a strided host-side sample; any
    row bias within ~(-87,+87) of the true row max is exact (fp32 range),
    and a host-side finite check retries with a bumped bias if ever violated.
  - KL is computed on-device from muT/lsT/alphasT (elementwise + reductions),
    chunked so it fills VectorE idle slots; the data-independent constant
    part of the KL sum is added back on the host.
"""

import numpy as np
import ml_dtypes

import concourse.bacc as bacc
import concourse.mybir as mybir
import concourse.tile as tile
from concourse import bass_utils

F32 = mybir.dt.float32
F16 = mybir.dt.float16
X = mybir.AxisListType.X
EXP = mybir.ActivationFunctionType.Exp

N_CORES = 8
T, K, R, V = 50, 50, 256, 50000
DELTA = 0.005
ROWS = T * K            # 2500
RP = 2560               # rows padded to 128 multiple
NB = RP // 128          # 20 row blocks
VP = 50176              # vocab padded; per-core 6272 = 12*512 + 128
VC = VP // N_CORES      # 6272 per-core vocab columns
KT = R // 128           # 2 contraction sub-tiles
CHUNK_SIZES = [2] * 10  # blocks per chunk (sum = NB)
KLCH = 500              # KL column chunk


def _vtiles(vc):
    ts = [512] * (vc // 512)
    if vc % 512:
        ts.append(vc % 512)
    return ts


def _vgroups(tiles):
    """Group consecutive v-tiles into <=4-tile (one PSUM slot) groups."""
    groups, i = [], 0
    while i < len(tiles):
        j = min(i + 4, len(tiles))
        groups.append((i, j))
        i = j
    return groups


def _emit(nc, tc, io):
    wth, wtl, ath, atl, muT, lsT, nbias, out, kl = io
    tiles = _vtiles(VC)
    offs = np.concatenate([[0], np.cumsum(tiles)]).tolist()
    groups = _vgroups(tiles)

    with (
        tc.tile_pool(name="wpool", bufs=1) as wp,
        tc.tile_pool(name="apool", bufs=1) as apool,
        tc.tile_pool(name="misc", bufs=1) as misc,
        tc.tile_pool(name="klp", bufs=2) as klp,
        tc.tile_pool(name="upool", bufs=4) as up,
        tc.tile_pool(name="pspool", bufs=2, space="PSUM") as pp,
        tc.tile_pool(name="stats", bufs=3) as stp,
        tc.tile_pool(name="ccp", bufs=2, space="DRAM") as ccp,
    ):
        # resident weights / alphas (fp16 hi+lo, per contraction sub-tile).
        # Alphas first (small, needed by every matmul), then W in column
        # chunks so the first vocab group's matmuls start early.
        wh = []
        wl = []
        ah = []
        al = []
        for kt in range(KT):
            t = apool.tile([128, RP], F16, tag=f"ah{kt}")
            nc.sync.dma_start(t[:], ath[kt * 128:(kt + 1) * 128, :])
            ah.append(t)
            t = apool.tile([128, RP], F16, tag=f"al{kt}")
            nc.sync.dma_start(t[:], atl[kt * 128:(kt + 1) * 128, :])
            al.append(t)
            twh = wp.tile([128, VC], F16, tag=f"wh{kt}")
            wh.append(twh)
            twl = wp.tile([128, VC], F16, tag=f"wl{kt}")
            wl.append(twl)
        nb = misc.tile([128, NB], F32, tag="nbias")
        nc.sync.dma_start(nb[:], nbias[:])
        for v0 in range(0, VC, 2048):
            v1 = min(v0 + 2048, VC)
            for kt in range(KT):
                nc.sync.dma_start(wh[kt][:, v0:v1],
                                  wth[kt * 128:(kt + 1) * 128, v0:v1])
                nc.sync.dma_start(wl[kt][:, v0:v1],
                                  wtl[kt * 128:(kt + 1) * 128, v0:v1])

        # ---- KL helpers (emitted interleaved with main chunks so the
        # DVE/ACT work lands in idle slots mid-run, not in the tail) ----
        kvec = misc.tile([128, 1], F32, tag="kvec")
        nc.vector.memset(kvec[:], 0.0)
        c0s = np.float32(1.0) / (np.float32(1.0) + np.float32(1e-6))
        log_delta = np.float32(np.log(np.float32(DELTA)))
        c1s = np.float32(1.0) / (np.exp(log_delta, dtype=np.float32) + np.float32(1e-6))

        def emit_kl_chunk(kt, c0):
            c1 = min(c0 + KLCH, ROWS)
            w = c1 - c0
            mu_t = klp.tile([128, KLCH], F32, tag="klmu")
            nc.sync.dma_start(mu_t[:, :w], muT[kt * 128:(kt + 1) * 128, c0:c1])
            ls_t = klp.tile([128, KLCH], F32, tag="klls")
            nc.sync.dma_start(ls_t[:, :w], lsT[kt * 128:(kt + 1) * 128, c0:c1])
            e = klp.tile([128, KLCH], F32, tag="kle")
            nc.scalar.activation(e[:, :w], ls_t[:, :w], EXP)
            # d = mu - p_alpha  (p_alpha = alphas shifted K cols; 0 for t=0)
            d = klp.tile([128, KLCH], F32, tag="kld")
            if c0 < K:
                nc.vector.tensor_copy(d[:, :K], mu_t[:, :K])
                nc.vector.tensor_sub(d[:, K:w], mu_t[:, K:w], ah[kt][:, 0:c1 - K])
                nc.vector.tensor_sub(d[:, K:w], d[:, K:w], al[kt][:, 0:c1 - K])
            else:
                nc.vector.tensor_sub(d[:, :w], mu_t[:, :w], ah[kt][:, c0 - K:c1 - K])
                nc.vector.tensor_sub(d[:, :w], d[:, :w], al[kt][:, c0 - K:c1 - K])
            nc.vector.tensor_mul(d[:, :w], d[:, :w], d[:, :w])
            nc.vector.tensor_add(e[:, :w], e[:, :w], d[:, :w])
            if c0 < K:
                nc.vector.tensor_scalar_mul(e[:, :K], e[:, :K], float(c0s))
                nc.vector.tensor_scalar_mul(e[:, K:w], e[:, K:w], float(c1s))
            else:
                nc.vector.tensor_scalar_mul(e[:, :w], e[:, :w], float(c1s))
            ra = klp.tile([128, 1], F32, tag="klra")
            nc.vector.reduce_sum(ra[:], e[:, :w], axis=X)
            rb = klp.tile([128, 1], F32, tag="klrb")
            nc.vector.reduce_sum(rb[:], ls_t[:, :w], axis=X)
            nc.vector.tensor_add(kvec[:], kvec[:], ra[:])
            nc.vector.tensor_sub(kvec[:], kvec[:], rb[:])

        kl_jobs = [(kt, c0) for kt in range(KT) for c0 in range(0, ROWS, KLCH)]

        # ---- main loop: matmul -> exp(+rowsum) -> allreduce -> scale -> out ----
        stationaries = []
        for kt in range(KT):
            stationaries.append((ah[kt], (wh[kt], wl[kt])))
            stationaries.append((al[kt], (wh[kt],)))
        n_acc = sum(len(m) for _, m in stationaries)  # 6 accumulating mms per slice

        # u is staged per block as two column-half tiles so slots recycle at
        # half-block granularity and scale/store pipeline per half.
        HSPLIT = (len(tiles) + 1) // 2  # v-tile index of the half boundary
        hoff = offs[HSPLIT]
        halves = [(0, HSPLIT, 0, hoff, "ua"),
                  (HSPLIT, len(tiles), hoff, VC - hoff, "ub")]
        hgroups = {}
        for h0, h1, hbase, hw, htag in halves:
            gs, i = [], h0
            while i < h1:
                j = min(i + 4, h1)
                gs.append((i, j))
                i = j
            hgroups[htag] = gs
        n_groups = sum(len(g) for g in hgroups.values())

        b0 = 0
        for ch, csz in enumerate(CHUNK_SIZES):
            uts = []
            sch = stp.tile([128, csz], F32, tag="sch")
            for bl in range(csz):
                b = b0 + bl
                blk_halves = {}
                sacc = stp.tile([128, n_groups], F32, tag="sacc")
                gi = 0
                for h0, h1, hbase, hw, htag in halves:
                    ut = up.tile([128, hw], F32, tag=htag)
                    blk_halves[htag] = ut
                    for g0, g1 in hgroups[htag]:
                        gw = offs[g1] - offs[g0]
                        pt = pp.tile([128, 2048], F32, tag="ps")
                        acc_i = 0
                        for sta, movs in stationaries:
                            for mov in movs:
                                for vt in range(g0, g1):
                                    o = offs[vt] - offs[g0]
                                    n = tiles[vt]
                                    nc.tensor.matmul(
                                        pt[:, o:o + n],
                                        sta[:, b * 128:(b + 1) * 128],
                                        mov[:, offs[vt]:offs[vt] + n],
                                        start=(acc_i == 0),
                                        stop=(acc_i == n_acc - 1),
                                    )
                                acc_i += 1
                        nc.scalar.activation(
                            ut[:, offs[g0] - hbase:offs[g0] - hbase + gw],
                            pt[:, :gw],
                            EXP,
                            bias=nb[:, b:b + 1],
                            scale=1.0,
                            accum_out=sacc[:, gi:gi + 1],
                        )
                        gi += 1
                uts.append(blk_halves)
                nc.vector.reduce_sum(sch[:, bl:bl + 1], sacc[:], axis=X)
            ccin = ccp.tile([128, csz], F32, tag="ccin")
            ccout = ccp.tile([128, csz], F32, tag="ccout")
            nc.gpsimd.dma_start(ccin[:], sch[:])
            nc.gpsimd.collective_compute(
                "AllReduce",
                mybir.AluOpType.add,
                replica_groups=[list(range(N_CORES))],
                ins=[ccin.opt()],
                outs=[ccout.opt()],
            )
            sg = stp.tile([128, csz], F32, tag="sg")
            nc.gpsimd.dma_start(sg[:], ccout[:])
            rec = stp.tile([128, csz], F32, tag="rec")
            nc.vector.reciprocal(rec[:], sg[:])
            for bl in range(csz):
                b = b0 + bl
                r0 = b * 128
                r1 = min(ROWS, r0 + 128)
                for h0, h1, hbase, hw, htag in halves:
                    ut = uts[bl][htag]
                    nc.vector.tensor_scalar_mul(ut[:], ut[:], rec[:, bl:bl + 1])
                    if r1 > r0:
                        nc.sync.dma_start(out[r0:r1, hbase:hbase + hw],
                                          ut[:r1 - r0, :])
            b0 += csz
            # keep KL work out of the busy head/tail windows
            if ch >= 2:
                for _ in range(2):
                    if kl_jobs:
                        emit_kl_chunk(*kl_jobs.pop(0))
        while kl_jobs:
            emit_kl_chunk(*kl_jobs.pop(0))

        # cross-partition sum of the KL vector via DRAM bounce
        kd = ccp.tile([128, 1], F32, tag="kldram")
        nc.sync.dma_start(kd[:], kvec[:])
        krow = misc.tile([1, 128], F32, tag="krow")
        nc.sync.dma_start(krow[0:1, :], kd[:].rearrange("a b -> b a"))
        ks = misc.tile([1, 1], F32, tag="ks")
        nc.vector.reduce_sum(ks[0:1, :], krow[0:1, :], axis=X)
        nc.sync.dma_start(kl[:], ks[0:1, 0:1])



_CACHE = {}


def _build():
    if "nc" in _CACHE:
        return _CACHE["nc"], _CACHE["io_names"]
    nc = bacc.Bacc("TRN2", target_bir_lowering=False, debug=False,
                   num_devices=N_CORES)
    wth = nc.dram_tensor("wth", [R, VC], F16, kind="ExternalInput").ap()
    wtl = nc.dram_tensor("wtl", [R, VC], F16, kind="ExternalInput").ap()
    ath = nc.dram_tensor("ath", [R, RP], F16, kind="ExternalInput").ap()
    atl = nc.dram_tensor("atl", [R, RP], F16, kind="ExternalInput").ap()
    muT = nc.dram_tensor("muT", [R, ROWS], F32, kind="ExternalInput").ap()
    lsT = nc.dram_tensor("lsT", [R, ROWS], F32, kind="ExternalInput").ap()
    nbias = nc.dram_tensor("nbias", [128, NB], F32, kind="ExternalInput").ap()
    out = nc.dram_tensor("out", [ROWS, VC], F32, kind="ExternalOutput").ap()
    kl = nc.dram_tensor("kl", [1, 1], F32, kind="ExternalOutput").ap()
    with tile.TileContext(nc) as tc:
        _emit(nc, tc, (wth, wtl, ath, atl, muT, lsT, nbias, out, kl))
    nc.compile()
    _CACHE["nc"] = nc
    _CACHE["io_names"] = None
    return nc, None


def _bf16_split(x):
    hi = x.astype(np.float16)
    lo = (x - hi.astype(np.float32)).astype(np.float16)
    return np.ascontiguousarray(hi), np.ascontiguousarray(lo)


def kernel(mu_q_alpha, logsigma_q_alpha, eps, W):
    mu = np.asarray(mu_q_alpha, dtype=np.float32)
    ls = np.asarray(logsigma_q_alpha, dtype=np.float32)
    ep = np.asarray(eps, dtype=np.float32)
    Wf = np.asarray(W, dtype=np.float32)

    # host prep: alphas ([T,K,R] row-major -> rows t*K+k), transposes, splits
    mu2 = np.transpose(mu, (1, 0, 2)).reshape(ROWS, R)
    ls2 = np.transpose(ls, (1, 0, 2)).reshape(ROWS, R)
    ep2 = ep.reshape(ROWS, R)
    alphas = mu2 + ep2 * np.exp(0.5 * ls2)

    ap = np.zeros((RP, R), np.float32)
    ap[:ROWS] = alphas
    aT = np.ascontiguousarray(ap.T)              # [256, 2560]
    ath, atl = _bf16_split(aT)
    muT = np.ascontiguousarray(mu2.T)            # [256, 2500]
    lsT = np.ascontiguousarray(ls2.T)

    Wp = np.zeros((VP, R), np.float32)
    Wp[:V] = Wf
    wt_h = []
    wt_l = []
    for c in range(N_CORES):
        Wc = np.ascontiguousarray(Wp[c * VC:(c + 1) * VC].T)  # [256, 6400]
        h, lo = _bf16_split(Wc)
        wt_h.append(h)
        wt_l.append(lo)

    # sampled per-row upper-ish bound for the exp shift (any value within
    # ~(-87,+87) of the true row max keeps fp32 exact; verified below)
    samp = alphas @ Wf[::499].T                  # [2500, ~101]
    brow_full = np.zeros(RP, np.float32)
    brow_full[:ROWS] = samp.max(axis=1)

    nc, _ = _build()

    for _attempt in range(4):
        nbias = np.ascontiguousarray(
            -brow_full.reshape(NB, 128).T.astype(np.float32))  # [128, NB]
        in_maps = []
        for c in range(N_CORES):
            in_maps.append({
                "wth": wt_h[c], "wtl": wt_l[c],
                "ath": ath, "atl": atl,
                "muT": muT, "lsT": lsT,
                "nbias": nbias,
            })
        res = None
        for _retry in range(4):
            try:
                res = bass_utils.run_bass_kernel_spmd(
                    nc, in_maps, core_ids=list(range(N_CORES)))
                break
            except Exception:
                if _retry == 3:
                    raise
                import time as _time
                _time.sleep(30.0 * (_retry + 1))
        beta2d = np.concatenate(
            [res.results[c]["out"] for c in range(N_CORES)], axis=1)[:, :V]
        # softmax rows must be finite and sum to ~1; a too-low bias overflows
        # exp (NaN row) or overflows the row-sum (all-zero row). Bump & retry.
        bad = (~np.isfinite(beta2d).all(axis=1)) | (
            np.abs(beta2d.sum(axis=1, dtype=np.float64) - 1.0) > 1e-3)
        if not bad.any():
            break
        brow_full[:ROWS][bad] += 60.0  # never hit for randn-scale inputs
    beta = beta2d.reshape(T, K, V)

    log_delta = np.float32(np.log(np.float32(DELTA)))
    c_const = float(R) * (K * (-1.0) + (ROWS - K) * (-1.0 + float(log_delta)))
    kl_alpha = np.float32(0.5 * (float(res.results[0]["kl"][0, 0]) + c_const))
    return beta, kl_alpha


if __name__ == "__main__":
    import reference

    inputs = {k: np.asarray(v) for k, v in reference.setup_inputs().items()}
    beta, kl_alpha = kernel(**inputs)
    print(beta.shape, beta.dtype, kl_alpha)
